# revision 47
# baseline (speedup 1.0000x reference)
"""Causal GQA attention on 8 TRN2 NeuronCores.

Problem: q [4096, 4096] = [bs*seq, 32 heads * 128], k/v [4096, 1024] =
[bs*seq, 8 kv heads * 128], causal softmax(q k^T / sqrt(128)) v with GQA
(4 query heads per kv head). f32 in/out.

Sharding: 8 cores = 2 batches x 4 head-groups. Each core owns one batch
and 8 query heads / 2 kv heads -- fully local, no collectives. Q and K are
handed to each core pre-permuted to [head_dim, head, seq] (host-side layout
marshalling in the shard step) so the contraction dim is already on
partitions; V is packed host-side with a fused ones column ([V_j | 1],
bf16) as the PV matmul wants it.

The kernel is organized around the ScalarE exp bottleneck (~139k PSUM
columns of exp per core at ~1 elem/cycle/lane): the S^T columns for all
(head, key-block) pairs form one continuous stream packed into rotating
[128, 1536] f32 PSUM regions (3 banks each, 2 regions = 6 banks), so every
exp is a single wide ACTIVATE (N=1536) instead of many narrow ones --
per-instruction overhead (~400ns) amortizes 12x better. The causal mask is
applied AFTER exp by a DVE multiply of each diagonal 128x128 P^T subtile
with a 0/1 lower-triangle (bf16 2x mode), keeping the QK->exp path free of
VectorE and making softmax denominators exact (masked probs are 0).

Per-core pipeline over regions r (91 per core):
  QK(r+1) on PE (K_j-stationary, <=512-wide bank-aligned chunks)
  || exp(r) on ScalarE (one ACTIVATE, scale folded in)
  || triangle masks(r) + chain normalizes on DVE
  || PV chains whose diagonal block lives in region r-1 on PE:
       acc[q,0:129] = sum_j P^T_j[:, s-subtile] @ [V_j | 1]
     (P^T-stationary accumulation in 2 rotating 1-bank PSUM tiles).

Walrus sync-wait limits (1 slot on DMA descriptors and LDWEIGHTS): all
loads land upfront in fresh buffers, tiny PE warmup matmuls absorb the
DMA semaphores into PE's vector clock (injected just before first use for
late-loaded pieces), and the triangle mask lives in a raw pre-Tile
preamble so it is dependency-free.

No max-subtraction softmax: logits are ~N(0,1) after scale, exp stays in
range; diag-block garbage (upper triangle) is finite and zeroed post-exp.
"""

import numpy as np

P = 128          # partitions / head_dim / key block
SEQ = 2048       # per-core sequence length
H = 8            # query heads per core
KV = 2           # kv heads per core
D = 128          # head dim
NB = SEQ // P    # 16 key blocks (also query subtiles) per head
SCALE = float(D) ** -0.5

REG_W = 1536     # S^T stream region width (3 PSUM banks of f32)
HEAD_W = sum(SEQ - P * j for j in range(NB))          # 17408 cols per head
STREAM_W = H * HEAD_W                                  # 139264 per core
NR = (STREAM_W + REG_W - 1) // REG_W                   # 91 regions

# Regions whose exp runs on the DVE (Schraudolph int-trick, see emit_act)
# instead of the saturated ScalarE. Measured: shifting exp into the DVE
# FIFO delays the PSUM st-region rotation (the DVE also carries chain
# copies/masks) and costs more in ACT-stream stalls than it saves, so the
# offload is disabled; the machinery is kept for experimentation.
DVE_EXP_REGIONS = frozenset()
LOG2E = 1.4426950408889634
SCH_A = float(2 ** 23 * LOG2E) * SCALE        # folds the 1/sqrt(d) scale
SCH_B = float(127 * 2 ** 23 - 550000)         # bias minus spline correction

_NC = None


def _block_base(h, j):
    """Stream position of the first column of (head h, key block j)."""
    return h * HEAD_W + SEQ * j - 64 * j * (j - 1)


def _stream_layout():
    """Build-time bookkeeping: region -> QK segments / diag subtiles, and
    (h, j, s) -> (region, offset) for PV chain stationary slices."""
    segs = [[] for _ in range(NR)]    # (h, j, q0, width, region_off)
    diags = [[] for _ in range(NR)]   # (h, j, region_off)
    for h in range(H):
        for j in range(NB):
            w = SEQ - P * j
            base = _block_base(h, j)
            r0, off0 = divmod(base, REG_W)
            diags[r0].append((h, j, off0))
            c = 0
            while c < w:
                r, off = divmod(base + c, REG_W)
                take = min(w - c, REG_W - off, 512 - (off % 512))
                segs[r].append((h, j, P * j + c, take, off))
                c += take
    return segs, diags


def _pt_slice_loc(h, j, s):
    """Region and offset of P^T_j[:, s-th 128-query subtile] for head h."""
    return divmod(_block_base(h, j) + P * (s - j), REG_W)


def _build_nc():
    import concourse.bass as bass
    import concourse.bacc as bacc
    import concourse.mybir as mybir
    import concourse.tile as tile
    from contextlib import ExitStack

    f32 = mybir.dt.float32
    bf16 = mybir.dt.bfloat16
    i32 = mybir.dt.int32
    i16 = mybir.dt.int16
    Exp = mybir.ActivationFunctionType.Exp
    Alu = mybir.AluOpType

    segs, diags = _stream_layout()

    nc = bacc.Bacc()
    qT_ext = nc.declare_dram_parameter("qT", [P, H, SEQ], bf16, isOutput=False)
    kT_ext = nc.declare_dram_parameter("kT", [P, KV, SEQ], bf16, isOutput=False)
    # vones ships pre-marshalled partition-major so the load is one DMA of
    # contiguous 8.25KB per partition (258-byte-descriptor loads take ~10us)
    v_ext = nc.declare_dram_parameter("vones", [P, NB * KV * (D + 1)], bf16,
                                      isOutput=False)
    tri_ext = nc.declare_dram_parameter("tri01", [P, P], bf16, isOutput=False)
    o_ext = nc.declare_dram_parameter("out", [SEQ, H * D], f32, isOutput=True)

    vd = v_ext.rearrange("p (i k c) -> p i k c", i=NB, k=KV)
    od = o_ext.rearrange("(i p) c -> p i c", p=P)

    with ExitStack() as ctx:
        tc = ctx.enter_context(tile.TileContext(nc))
        singles = ctx.enter_context(tc.tile_pool(name="singles", bufs=1))
        pt_pool = ctx.enter_context(tc.tile_pool(name="pt", bufs=20))
        ob_pool = ctx.enter_context(tc.tile_pool(name="ob", bufs=2))
        r_pool = ctx.enter_context(tc.tile_pool(name="r", bufs=8))
        z_pool = ctx.enter_context(tc.tile_pool(name="z", bufs=2))
        ps_st = ctx.enter_context(tc.tile_pool(name="ps_st", bufs=2, space="PSUM"))
        ps_pv = ctx.enter_context(tc.tile_pool(name="ps_pv", bufs=2, space="PSUM"))

        # ---- upfront loads, each into a fresh buffer on a fresh queue ----
        # Head 0 / kv 0 pieces come first so compute starts early.
        kt = singles.tile([P, KV, SEQ], bf16)        # [d, kv, key]
        qt = singles.tile([P, H, SEQ], bf16)         # [d, head, query]
        vones = singles.tile([P, NB, KV, D + 1], bf16)  # [k, block, kv, d|1]
        # Loads are split across the SP (sync) and GpSimd trigger queues:
        # each DMA trigger costs ~0.7us of serial queue time, so the
        # critical pieces (triangle, kt/qt of head 0) sit at the front of
        # one queue while vones rides the other.
        # One queue, strict need-order: concurrent queues split HBM
        # bandwidth and starve the critical head-0 pieces. vones rides after
        # kt0b -- the budgeted chain backlog absorbs its late arrival.
        tri01 = singles.tile([P, P], bf16)
        nc.sync.dma_start(out=tri01, in_=tri_ext.ap())
        nc.sync.dma_start(out=kt[:, 0, 0:128], in_=kT_ext.ap()[:, 0, 0:128])
        nc.sync.dma_start(out=qt[:, 0, 0:1536], in_=qT_ext.ap()[:, 0, 0:1536])
        nc.sync.dma_start(out=kt[:, 0, 128:512], in_=kT_ext.ap()[:, 0, 128:512])
        nc.sync.dma_start(out=qt[:, 0, 1536:], in_=qT_ext.ap()[:, 0, 1536:])
        nc.sync.dma_start(out=kt[:, 0, 512:], in_=kT_ext.ap()[:, 0, 512:])
        nc.sync.dma_start(out=vones, in_=vd)
        nc.sync.dma_start(out=qt[:, 1:4, :], in_=qT_ext.ap()[:, 1:4, :])
        nc.sync.dma_start(out=kt[:, 1:2, :], in_=kT_ext.ap()[:, 1:2, :])
        nc.sync.dma_start(out=qt[:, 4:8, :], in_=qT_ext.ap()[:, 4:8, :])

        # Scratch initialized instantly by DVE at t=0: the exp table-load
        # warmup and HAM pre-warm run on it with no DMA dependency.
        scratch = singles.tile([P, P], f32)
        nc.vector.memset(scratch, 0.5)

        # ---- PE warmups: absorb DMA/DVE semaphores into PE's clock so real
        # matmuls never carry a second wait. Outputs unread.
        def warm(ap):
            wm = ps_pv.tile([2, 2], f32, tag="pvacc", name="wm")
            nc.tensor.matmul(wm[:1, :1], lhsT=ap, rhs=ap, start=True, stop=True)

        # HAM pre-warm: ~2.5us of back-to-back dummy matmuls so the PE clock
        # gate opens before the first real QK burst instead of ~20us in.
        # These run while the DMAs land.
        hamwm = ps_pv.tile([P, P], f32, tag="pvacc", name="hamwm")
        scr16 = scratch.bitcast(bf16)
        for _ in range(48):
            nc.tensor.matmul(hamwm, lhsT=scr16[:, 0:P], rhs=scr16[:, 0:P],
                             start=True, stop=True)

        warm(tri01[:, 0:1])
        warm(kt[:, 0, 0:1])
        warm(qt[:, 0, 0:1])
        # exp table load early, overlapping the remaining DMAs
        actwarm = singles.tile([P, P], bf16)
        nc.scalar.activation(out=actwarm, in_=scratch, func=Exp, scale=SCALE)

        # one warm per remaining DMA piece, injected just before first use
        # (an upfront warm would park the PE FIFO until that DMA lands)
        warm_aps = {
            "q0b": qt[:, 0, 1536:1537],
            "q123": qt[:, 1, 0:1],
            "q4567": qt[:, 4, 0:1],
            "k0a2": kt[:, 0, 128:129],
            "k0b": kt[:, 0, 512:513],
            "k1": kt[:, 1, 0:1],
            "vones": vones[:, 0, 0, 0:1],
        }
        warmed = set()

        def warm_piece(piece):
            if piece and piece not in warmed:
                warmed.add(piece)
                warm(warm_aps[piece])

        def q_piece(h, q1):
            if h == 0:
                return "q0b" if q1 > 1536 else None
            return "q123" if h < 4 else "q4567"

        def k_piece(kvh, j):
            if kvh == 0:
                if j == 0:
                    return None
                return "k0a2" if j < 4 else "k0b"
            return "k1"

        # ---- pipelined region loop ----
        st_tiles = {}
        pt_tiles = {}
        o_sbs = {}

        def emit_qk(r):
            st = ps_st.tile([P, REG_W], f32, name="st")
            st_tiles[r] = st
            for (h, j, q0, w, off) in segs[r]:
                kvh = h // (H // KV)
                warm_piece(q_piece(h, q0 + w))
                warm_piece(k_piece(kvh, j))
                nc.tensor.matmul(
                    st[:, off:off + w],
                    lhsT=kt[:, kvh, j * P:(j + 1) * P],
                    rhs=qt[:, h, q0:q0 + w],
                    start=True,
                    stop=True,
                )

        def emit_act(r):
            w = min(REG_W, STREAM_W - r * REG_W)
            pt = pt_pool.tile([P, REG_W], bf16, name="pt")
            pt_tiles[r] = pt
            st = st_tiles.pop(r)
            if r in DVE_EXP_REGIONS:
                # exp on the DVE: z = round(S * 2^23 log2(e) / sqrt(d) + B)
                # as int32; the high 16 bits of z ARE the bf16 pattern of
                # ~exp(S/sqrt(d)) (linear-mantissa approximation).
                z = z_pool.tile([P, REG_W], i32, name="z")
                nc.vector.tensor_scalar(
                    out=z[:, 0:w], in0=st[:, 0:w],
                    scalar1=SCH_A, scalar2=SCH_B,
                    op0=Alu.mult, op1=Alu.add,
                )
                z_hi = z[:, 0:w].bitcast(i16).rearrange(
                    "p (w two) -> p w two", two=2)[:, :, 1]
                nc.vector.tensor_copy(out=pt[:, 0:w].bitcast(i16), in_=z_hi)
            else:
                nc.scalar.activation(
                    out=pt[:, 0:w], in_=st[:, 0:w], func=Exp, scale=SCALE,
                )

        def emit_masks(r):
            pt = pt_tiles[r]
            for (h, j, off) in diags[r]:
                nc.vector.tensor_mul(
                    out=pt[:, off:off + P],
                    in0=pt[:, off:off + P],
                    in1=tri01,
                )

        # PV accumulators are PAIRED two-to-a-bank: chains (h, s even) and
        # (h, s odd) write halves of one [P, 2, 129] PSUM tile, drained by a
        # single DVE copy. PE never writes a bank the DVE still reads
        # (pair-granularity rotation), copies halve, and the acc-WAR slack
        # doubles (4 chains instead of 2).
        pair_state = {}  # "tile" -> open pair tile for (h, even s)

        def flush_pair(h, s_hi, n):
            acc = pair_state.pop("tile")
            o_raw = o_sbs[h]
            s_lo = s_hi - n + 1
            nc.vector.tensor_copy(out=o_raw[:, s_lo:s_hi + 1, :],
                                  in_=acc[:, 0:n, :])
            # last head: normalize+store per pair to shorten the kernel tail
            grp = 2 if h == H - 1 else 4
            if (s_hi + 1) % grp == 0:
                g0 = s_hi - grp + 1
                rcp = r_pool.tile([P, 4], f32, name="rcp")
                nc.vector.reciprocal(rcp[:, 0:grp], o_raw[:, g0:s_hi + 1, D])
                for i in range(grp):
                    si = g0 + i
                    nc.vector.tensor_scalar_mul(
                        o_raw[:, si, 0:D], o_raw[:, si, 0:D], rcp[:, i:i + 1]
                    )
                nc.sync.dma_start(
                    out=od[:, g0:s_hi + 1, h * D:(h + 1) * D],
                    in_=o_raw[:, g0:s_hi + 1, 0:D],
                )

        def emit_chain(h, s):
            kvh = h // (H // KV)
            warm_piece("vones")
            if s == 0:
                o_sbs[h] = ob_pool.tile([P, NB, D + 1], f32, name="o_raw")
            if "tile" not in pair_state:
                pair_state["tile"] = ps_pv.tile([P, 2, D + 1], f32,
                                                tag="pvacc", name="pvacc")
            acc = pair_state["tile"][:, s % 2, :]
            for j in range(s + 1):
                rr, off = _pt_slice_loc(h, j, s)
                nc.tensor.matmul(
                    acc,
                    lhsT=pt_tiles[rr][:, off:off + P],
                    rhs=vones[:, j, kvh, :],
                    start=(j == 0),
                    stop=(j == s),
                )
            if s % 2 == 1:
                flush_pair(h, s, 2)

        # Iteration r: ACT(r) [needs QK(r), emitted 2 iters ago]; chains for
        # diag region r-1 (runnable: only need ACT(r-1)+mask(r-1)) BEFORE
        # QK(r+2) (gated on ACT(r) via the st WAR) so the PE FIFO never
        # parks runnable chain work behind the region gate. On the DVE FIFO
        # the chain copies/normalizes (gated only on PE) come BEFORE
        # masks(r) (gated on ACT(r)) so they never wait behind it.
        # Chain release is smoothed: diagonal blocks cluster at head tails
        # (block widths shrink toward j=15, so several long chains become
        # runnable in the last regions of a head). A pending queue caps the
        # chain work emitted per iteration (~BUDGET PV steps ~= one ACTIVATE
        # of PE time) and spills the excess into the chain-light early
        # regions of the next head.
        # Chains release 2 regions behind the ACT stream (the PE runs ~1
        # region ahead, so the diag subtile's DVE mask is already done when
        # the chain's last step loads it); at the very tail they release as
        # fresh as possible.
        BUDGET = 16
        pending = []
        released = 0
        for rr in range(min(2, NR)):
            emit_qk(rr)
        for r in range(NR):
            emit_act(r)
            target = r - 1 if r >= NR - 2 else r - 2
            while released <= target:
                pending.extend((h, j) for (h, j, _off) in diags[released])
                released += 1
            steps = 0
            budget = BUDGET if r < NR - 8 else 10 ** 9
            while pending and steps < budget:
                h, s = pending.pop(0)
                emit_chain(h, s)
                steps += s + 1
            emit_masks(r)
            if r + 2 < NR:
                emit_qk(r + 2)
        while released < NR:
            pending.extend((h, j) for (h, j, _off) in diags[released])
            released += 1
        for (h, s) in pending:
            emit_chain(h, s)

    nc.compile()
    return nc


def _get_nc():
    global _NC
    if _NC is None:
        _NC = _build_nc()
    return _NC


def _shard_inputs(q, k, v):
    import ml_dtypes
    in_maps = []
    ones = np.ones((SEQ, KV, 1), np.float32)
    # keep P^T[k, q_local] where q_local >= k
    tri01 = np.triu(np.ones((P, P), np.float32)).astype(ml_dtypes.bfloat16)
    for c in range(8):
        b, hg = divmod(c, 4)
        rs = slice(b * SEQ, (b + 1) * SEQ)
        qs = q[rs, hg * 1024:(hg + 1) * 1024]    # [seq, 8*128]
        ks = k[rs, hg * 256:(hg + 1) * 256]      # [seq, 2*128]
        vs = v[rs, hg * 256:(hg + 1) * 256].reshape(SEQ, KV, D)
        # partition-major: vo[p, i, kv, d|1] for seq = i*128 + p
        vo = np.concatenate([vs, ones], axis=2).reshape(NB, P, KV * (D + 1))
        vo = vo.transpose(1, 0, 2).reshape(P, NB * KV * (D + 1))
        in_maps.append({
            "qT": np.ascontiguousarray(
                qs.reshape(SEQ, H, D).transpose(2, 1, 0)
            ).astype(ml_dtypes.bfloat16),
            "kT": np.ascontiguousarray(
                ks.reshape(SEQ, KV, D).transpose(2, 1, 0)
            ).astype(ml_dtypes.bfloat16),
            "vones": np.ascontiguousarray(vo).astype(ml_dtypes.bfloat16),
            "tri01": tri01,
        })
    return in_maps


def _run(q, k, v, **spmd_kwargs):
    from concourse.bass_utils import run_bass_kernel_spmd

    nc = _get_nc()
    bkr = run_bass_kernel_spmd(nc, _shard_inputs(q, k, v),
                               core_ids=list(range(8)), **spmd_kwargs)
    out = np.empty((2 * SEQ, 32 * D), np.float32)
    for c in range(8):
        b, hg = divmod(c, 4)
        out[b * SEQ:(b + 1) * SEQ, hg * 1024:(hg + 1) * 1024] = \
            bkr.results[c]["out"]
    return out, bkr


def kernel(q, k, v, bs=2, seq_len=2048, **_ignored):
    q = np.asarray(q, dtype=np.float32)
    k = np.asarray(k, dtype=np.float32)
    v = np.asarray(v, dtype=np.float32)
    assert int(bs) == 2 and int(seq_len) == SEQ
    assert q.shape == (4096, 4096) and k.shape == (4096, 1024)
    out, _ = _run(q, k, v)
    return out


# revision 48
# speedup vs baseline: 1.1723x; 1.1723x over previous
"""Causal GQA attention on 8 TRN2 NeuronCores.

Problem: q [4096, 4096] = [bs*seq, 32 heads * 128], k/v [4096, 1024] =
[bs*seq, 8 kv heads * 128], causal softmax(q k^T / sqrt(128)) v with GQA
(4 query heads per kv head). f32 in/out.

Sharding: 8 cores = 2 batches x 4 head-groups. Each core owns one batch
and 8 query heads / 2 kv heads -- fully local, no collectives. Q and K are
handed to each core pre-permuted to [head_dim, head, seq] (host-side layout
marshalling in the shard step) so the contraction dim is already on
partitions; V is packed host-side with a fused ones column ([V_j | 1],
bf16) as the PV matmul wants it.

The kernel is organized around the ScalarE exp bottleneck (~139k PSUM
columns of exp per core at ~1 elem/cycle/lane): the S^T columns for all
(head, key-block) pairs form one continuous stream packed into rotating
[128, 1536] f32 PSUM regions (3 banks each, 2 regions = 6 banks), so every
exp is a single wide ACTIVATE (N=1536) instead of many narrow ones --
per-instruction overhead (~400ns) amortizes 12x better. The causal mask is
applied AFTER exp by a DVE multiply of each diagonal 128x128 P^T subtile
with a 0/1 lower-triangle (bf16 2x mode), keeping the QK->exp path free of
VectorE and making softmax denominators exact (masked probs are 0).

Per-core pipeline over regions r (91 per core):
  QK(r+2) on PE (K_j-stationary, <=512-wide bank-aligned chunks)
  || exp(r) on ScalarE (one ACTIVATE, scale folded in)
  || triangle masks(r) + paired chain drains on DVE
  || PV chains released 2 regions behind the ACT stream on PE:
       acc[q,0:129] = sum_j P^T_j[:, s-subtile] @ [V_j | 1]
     (P^T-stationary accumulation; two chains share one 1-bank PSUM tile
     and drain with a single DVE copy, so the accumulator WAR has 4 chains
     of slack; normalization runs from SBUF in batches of 4 subtiles).
Chain release is budgeted (~16 PV steps/region) to smooth the diag-block
clustering at head tails into the chain-light early regions of the next
head. DMA loads ride one HWDGE queue in strict first-use order (parallel
queues split HBM bandwidth and starve the critical head-0 pieces); a
~4us burst of dummy matmuls opens the PE HAM clock gate while they land.

Walrus sync-wait limits (1 slot on DMA descriptors and LDWEIGHTS): tiny
PE warmup matmuls absorb each DMA piece's semaphore into PE's vector
clock, injected just before the piece's first use so they never park the
PE FIFO on a DMA that hasn't landed.

No max-subtraction softmax: logits are ~N(0,1) after scale, exp stays in
range; diag-block garbage (upper triangle) is finite and zeroed post-exp.
"""

import numpy as np

P = 128          # partitions / head_dim / key block
SEQ = 2048       # per-core sequence length
H = 8            # query heads per core
KV = 2           # kv heads per core
D = 128          # head dim
NB = SEQ // P    # 16 key blocks (also query subtiles) per head
SCALE = float(D) ** -0.5

REG_W = 1536     # S^T stream region width (3 PSUM banks of f32)
HEAD_W = sum(SEQ - P * j for j in range(NB))          # 17408 cols per head
STREAM_W = H * HEAD_W                                  # 139264 per core
NR = (STREAM_W + REG_W - 1) // REG_W                   # 91 regions

# Regions whose exp runs on the DVE (Schraudolph int-trick, see emit_act)
# instead of the saturated ScalarE. Measured: shifting exp into the DVE
# FIFO delays the PSUM st-region rotation (the DVE also carries chain
# copies/masks) and costs more in ACT-stream stalls than it saves, so the
# offload is disabled; the machinery is kept for experimentation.
DVE_EXP_REGIONS = frozenset()
LOG2E = 1.4426950408889634
SCH_A = float(2 ** 23 * LOG2E) * SCALE        # folds the 1/sqrt(d) scale
SCH_B = float(127 * 2 ** 23 - 550000)         # bias minus spline correction

_NC = None


def _block_base(h, j):
    """Stream position of the first column of (head h, key block j)."""
    return h * HEAD_W + SEQ * j - 64 * j * (j - 1)


def _stream_layout():
    """Build-time bookkeeping: region -> QK segments / diag subtiles, and
    (h, j, s) -> (region, offset) for PV chain stationary slices."""
    segs = [[] for _ in range(NR)]    # (h, j, q0, width, region_off)
    diags = [[] for _ in range(NR)]   # (h, j, region_off)
    for h in range(H):
        for j in range(NB):
            w = SEQ - P * j
            base = _block_base(h, j)
            r0, off0 = divmod(base, REG_W)
            diags[r0].append((h, j, off0))
            c = 0
            while c < w:
                r, off = divmod(base + c, REG_W)
                take = min(w - c, REG_W - off, 512 - (off % 512))
                segs[r].append((h, j, P * j + c, take, off))
                c += take
    return segs, diags


def _pt_slice_loc(h, j, s):
    """Region and offset of P^T_j[:, s-th 128-query subtile] for head h."""
    return divmod(_block_base(h, j) + P * (s - j), REG_W)


def _build_nc():
    import concourse.bass as bass
    import concourse.bacc as bacc
    import concourse.mybir as mybir
    import concourse.tile as tile
    from contextlib import ExitStack

    f32 = mybir.dt.float32
    bf16 = mybir.dt.bfloat16
    i32 = mybir.dt.int32
    i16 = mybir.dt.int16
    Exp = mybir.ActivationFunctionType.Exp
    Alu = mybir.AluOpType

    segs, diags = _stream_layout()

    nc = bacc.Bacc()
    qT_ext = nc.declare_dram_parameter("qT", [P, H, SEQ], bf16, isOutput=False)
    kT_ext = nc.declare_dram_parameter("kT", [P, KV, SEQ], bf16, isOutput=False)
    # vones ships pre-marshalled partition-major so the load is one DMA of
    # contiguous 8.25KB per partition (258-byte-descriptor loads take ~10us)
    v_ext = nc.declare_dram_parameter("vones", [P, NB * KV * (D + 1)], bf16,
                                      isOutput=False)
    tri_ext = nc.declare_dram_parameter("tri01", [P, P], bf16, isOutput=False)
    o_ext = nc.declare_dram_parameter("out", [SEQ, H * D], f32, isOutput=True)

    vd = v_ext.rearrange("p (i k c) -> p i k c", i=NB, k=KV)
    od = o_ext.rearrange("(i p) c -> p i c", p=P)

    with ExitStack() as ctx:
        tc = ctx.enter_context(tile.TileContext(nc))
        singles = ctx.enter_context(tc.tile_pool(name="singles", bufs=1))
        pt_pool = ctx.enter_context(tc.tile_pool(name="pt", bufs=20))
        ob_pool = ctx.enter_context(tc.tile_pool(name="ob", bufs=2))
        r_pool = ctx.enter_context(tc.tile_pool(name="r", bufs=8))
        z_pool = ctx.enter_context(tc.tile_pool(name="z", bufs=2))
        ps_st = ctx.enter_context(tc.tile_pool(name="ps_st", bufs=2, space="PSUM"))
        ps_pv = ctx.enter_context(tc.tile_pool(name="ps_pv", bufs=2, space="PSUM"))

        # ---- upfront loads, each into a fresh buffer on a fresh queue ----
        # Head 0 / kv 0 pieces come first so compute starts early.
        kt = singles.tile([P, KV, SEQ], bf16)        # [d, kv, key]
        qt = singles.tile([P, H, SEQ], bf16)         # [d, head, query]
        vones = singles.tile([P, NB, KV, D + 1], bf16)  # [k, block, kv, d|1]
        # Loads are split across the SP (sync) and GpSimd trigger queues:
        # each DMA trigger costs ~0.7us of serial queue time, so the
        # critical pieces (triangle, kt/qt of head 0) sit at the front of
        # one queue while vones rides the other.
        # One queue, strict need-order: concurrent queues split HBM
        # bandwidth and starve the critical head-0 pieces. vones rides after
        # kt0b -- the budgeted chain backlog absorbs its late arrival.
        tri01 = singles.tile([P, P], bf16)
        nc.sync.dma_start(out=tri01, in_=tri_ext.ap())
        nc.sync.dma_start(out=kt[:, 0, 0:128], in_=kT_ext.ap()[:, 0, 0:128])
        nc.sync.dma_start(out=qt[:, 0, 0:1536], in_=qT_ext.ap()[:, 0, 0:1536])
        nc.sync.dma_start(out=kt[:, 0, 128:512], in_=kT_ext.ap()[:, 0, 128:512])
        nc.sync.dma_start(out=qt[:, 0, 1536:], in_=qT_ext.ap()[:, 0, 1536:])
        nc.sync.dma_start(out=kt[:, 0, 512:], in_=kT_ext.ap()[:, 0, 512:])
        nc.sync.dma_start(out=vones, in_=vd)
        nc.sync.dma_start(out=qt[:, 1:4, :], in_=qT_ext.ap()[:, 1:4, :])
        nc.sync.dma_start(out=kt[:, 1:2, :], in_=kT_ext.ap()[:, 1:2, :])
        nc.sync.dma_start(out=qt[:, 4:8, :], in_=qT_ext.ap()[:, 4:8, :])

        # Scratch initialized instantly by DVE at t=0: the exp table-load
        # warmup and HAM pre-warm run on it with no DMA dependency.
        scratch = singles.tile([P, P], f32)
        nc.vector.memset(scratch, 0.5)

        # ---- PE warmups: absorb DMA/DVE semaphores into PE's clock so real
        # matmuls never carry a second wait. Outputs unread.
        def warm(ap):
            wm = ps_pv.tile([2, 2], f32, tag="pvacc", name="wm")
            nc.tensor.matmul(wm[:1, :1], lhsT=ap, rhs=ap, start=True, stop=True)

        # HAM pre-warm: ~2.5us of back-to-back dummy matmuls so the PE clock
        # gate opens before the first real QK burst instead of ~20us in.
        # These run while the DMAs land.
        hamwm = ps_pv.tile([P, P], f32, tag="pvacc", name="hamwm")
        scr16 = scratch.bitcast(bf16)
        for _ in range(48):
            nc.tensor.matmul(hamwm, lhsT=scr16[:, 0:P], rhs=scr16[:, 0:P],
                             start=True, stop=True)

        warm(tri01[:, 0:1])
        warm(kt[:, 0, 0:1])
        warm(qt[:, 0, 0:1])
        # exp table load early, overlapping the remaining DMAs
        actwarm = singles.tile([P, P], bf16)
        nc.scalar.activation(out=actwarm, in_=scratch, func=Exp, scale=SCALE)

        # one warm per remaining DMA piece, injected just before first use
        # (an upfront warm would park the PE FIFO until that DMA lands)
        warm_aps = {
            "q0b": qt[:, 0, 1536:1537],
            "q123": qt[:, 1, 0:1],
            "q4567": qt[:, 4, 0:1],
            "k0a2": kt[:, 0, 128:129],
            "k0b": kt[:, 0, 512:513],
            "k1": kt[:, 1, 0:1],
            "vones": vones[:, 0, 0, 0:1],
        }
        warmed = set()

        def warm_piece(piece):
            if piece and piece not in warmed:
                warmed.add(piece)
                warm(warm_aps[piece])

        def q_piece(h, q1):
            if h == 0:
                return "q0b" if q1 > 1536 else None
            return "q123" if h < 4 else "q4567"

        def k_piece(kvh, j):
            if kvh == 0:
                if j == 0:
                    return None
                return "k0a2" if j < 4 else "k0b"
            return "k1"

        # ---- pipelined region loop ----
        st_tiles = {}
        pt_tiles = {}
        o_sbs = {}

        def emit_qk(r):
            st = ps_st.tile([P, REG_W], f32, name="st")
            st_tiles[r] = st
            for (h, j, q0, w, off) in segs[r]:
                kvh = h // (H // KV)
                warm_piece(q_piece(h, q0 + w))
                warm_piece(k_piece(kvh, j))
                nc.tensor.matmul(
                    st[:, off:off + w],
                    lhsT=kt[:, kvh, j * P:(j + 1) * P],
                    rhs=qt[:, h, q0:q0 + w],
                    start=True,
                    stop=True,
                )

        def emit_act(r):
            w = min(REG_W, STREAM_W - r * REG_W)
            pt = pt_pool.tile([P, REG_W], bf16, name="pt")
            pt_tiles[r] = pt
            st = st_tiles.pop(r)
            if r in DVE_EXP_REGIONS:
                # exp on the DVE: z = round(S * 2^23 log2(e) / sqrt(d) + B)
                # as int32; the high 16 bits of z ARE the bf16 pattern of
                # ~exp(S/sqrt(d)) (linear-mantissa approximation).
                z = z_pool.tile([P, REG_W], i32, name="z")
                nc.vector.tensor_scalar(
                    out=z[:, 0:w], in0=st[:, 0:w],
                    scalar1=SCH_A, scalar2=SCH_B,
                    op0=Alu.mult, op1=Alu.add,
                )
                z_hi = z[:, 0:w].bitcast(i16).rearrange(
                    "p (w two) -> p w two", two=2)[:, :, 1]
                nc.vector.tensor_copy(out=pt[:, 0:w].bitcast(i16), in_=z_hi)
            else:
                nc.scalar.activation(
                    out=pt[:, 0:w], in_=st[:, 0:w], func=Exp, scale=SCALE,
                )

        def emit_masks(r):
            pt = pt_tiles[r]
            for (h, j, off) in diags[r]:
                nc.vector.tensor_mul(
                    out=pt[:, off:off + P],
                    in0=pt[:, off:off + P],
                    in1=tri01,
                )

        # PV accumulators are PAIRED two-to-a-bank: chains (h, s even) and
        # (h, s odd) write halves of one [P, 2, 129] PSUM tile, drained by a
        # single DVE copy. PE never writes a bank the DVE still reads
        # (pair-granularity rotation), copies halve, and the acc-WAR slack
        # doubles (4 chains instead of 2).
        pair_state = {}  # "tile" -> open pair tile for (h, even s)

        def flush_pair(h, s_hi, n):
            acc = pair_state.pop("tile")
            o_raw = o_sbs[h]
            s_lo = s_hi - n + 1
            nc.vector.tensor_copy(out=o_raw[:, s_lo:s_hi + 1, :],
                                  in_=acc[:, 0:n, :])
            # last head: normalize+store per pair to shorten the kernel tail
            grp = 2 if h == H - 1 else 4
            if (s_hi + 1) % grp == 0:
                g0 = s_hi - grp + 1
                rcp = r_pool.tile([P, 4], f32, name="rcp")
                nc.vector.reciprocal(rcp[:, 0:grp], o_raw[:, g0:s_hi + 1, D])
                for i in range(grp):
                    si = g0 + i
                    nc.vector.tensor_scalar_mul(
                        o_raw[:, si, 0:D], o_raw[:, si, 0:D], rcp[:, i:i + 1]
                    )
                nc.sync.dma_start(
                    out=od[:, g0:s_hi + 1, h * D:(h + 1) * D],
                    in_=o_raw[:, g0:s_hi + 1, 0:D],
                )

        def emit_chain(h, s):
            kvh = h // (H // KV)
            warm_piece("vones")
            if s == 0:
                o_sbs[h] = ob_pool.tile([P, NB, D + 1], f32, name="o_raw")
            if "tile" not in pair_state:
                pair_state["tile"] = ps_pv.tile([P, 2, D + 1], f32,
                                                tag="pvacc", name="pvacc")
            acc = pair_state["tile"][:, s % 2, :]
            for j in range(s + 1):
                rr, off = _pt_slice_loc(h, j, s)
                nc.tensor.matmul(
                    acc,
                    lhsT=pt_tiles[rr][:, off:off + P],
                    rhs=vones[:, j, kvh, :],
                    start=(j == 0),
                    stop=(j == s),
                )
            if s % 2 == 1:
                flush_pair(h, s, 2)

        # Iteration r: ACT(r) [needs QK(r), emitted 2 iters ago]; chains for
        # diag region r-1 (runnable: only need ACT(r-1)+mask(r-1)) BEFORE
        # QK(r+2) (gated on ACT(r) via the st WAR) so the PE FIFO never
        # parks runnable chain work behind the region gate. On the DVE FIFO
        # the chain copies/normalizes (gated only on PE) come BEFORE
        # masks(r) (gated on ACT(r)) so they never wait behind it.
        # Chain release is smoothed: diagonal blocks cluster at head tails
        # (block widths shrink toward j=15, so several long chains become
        # runnable in the last regions of a head). A pending queue caps the
        # chain work emitted per iteration (~BUDGET PV steps ~= one ACTIVATE
        # of PE time) and spills the excess into the chain-light early
        # regions of the next head.
        # Chains release 2 regions behind the ACT stream (the PE runs ~1
        # region ahead, so the diag subtile's DVE mask is already done when
        # the chain's last step loads it); at the very tail they release as
        # fresh as possible.
        BUDGET = 16
        pending = []
        released = 0
        for rr in range(min(2, NR)):
            emit_qk(rr)
        for r in range(NR):
            emit_act(r)
            target = r - 1 if r >= NR - 2 else r - 2
            while released <= target:
                pending.extend((h, j) for (h, j, _off) in diags[released])
                released += 1
            steps = 0
            budget = BUDGET if r < NR - 8 else 10 ** 9
            while pending and steps < budget:
                h, s = pending.pop(0)
                emit_chain(h, s)
                steps += s + 1
            emit_masks(r)
            if r + 2 < NR:
                emit_qk(r + 2)
        while released < NR:
            pending.extend((h, j) for (h, j, _off) in diags[released])
            released += 1
        for (h, s) in pending:
            emit_chain(h, s)

    nc.compile()
    return nc


def _get_nc():
    global _NC
    if _NC is None:
        _NC = _build_nc()
    return _NC


def _shard_inputs(q, k, v):
    import ml_dtypes
    in_maps = []
    ones = np.ones((SEQ, KV, 1), np.float32)
    # keep P^T[k, q_local] where q_local >= k
    tri01 = np.triu(np.ones((P, P), np.float32)).astype(ml_dtypes.bfloat16)
    for c in range(8):
        b, hg = divmod(c, 4)
        rs = slice(b * SEQ, (b + 1) * SEQ)
        qs = q[rs, hg * 1024:(hg + 1) * 1024]    # [seq, 8*128]
        ks = k[rs, hg * 256:(hg + 1) * 256]      # [seq, 2*128]
        vs = v[rs, hg * 256:(hg + 1) * 256].reshape(SEQ, KV, D)
        # partition-major: vo[p, i, kv, d|1] for seq = i*128 + p
        vo = np.concatenate([vs, ones], axis=2).reshape(NB, P, KV * (D + 1))
        vo = vo.transpose(1, 0, 2).reshape(P, NB * KV * (D + 1))
        in_maps.append({
            "qT": np.ascontiguousarray(
                qs.reshape(SEQ, H, D).transpose(2, 1, 0)
            ).astype(ml_dtypes.bfloat16),
            "kT": np.ascontiguousarray(
                ks.reshape(SEQ, KV, D).transpose(2, 1, 0)
            ).astype(ml_dtypes.bfloat16),
            "vones": np.ascontiguousarray(vo).astype(ml_dtypes.bfloat16),
            "tri01": tri01,
        })
    return in_maps


def _run(q, k, v, **spmd_kwargs):
    from concourse.bass_utils import run_bass_kernel_spmd

    nc = _get_nc()
    bkr = run_bass_kernel_spmd(nc, _shard_inputs(q, k, v),
                               core_ids=list(range(8)), **spmd_kwargs)
    out = np.empty((2 * SEQ, 32 * D), np.float32)
    for c in range(8):
        b, hg = divmod(c, 4)
        out[b * SEQ:(b + 1) * SEQ, hg * 1024:(hg + 1) * 1024] = \
            bkr.results[c]["out"]
    return out, bkr


def kernel(q, k, v, bs=2, seq_len=2048, **_ignored):
    q = np.asarray(q, dtype=np.float32)
    k = np.asarray(k, dtype=np.float32)
    v = np.asarray(v, dtype=np.float32)
    assert int(bs) == 2 and int(seq_len) == SEQ
    assert q.shape == (4096, 4096) and k.shape == (4096, 1024)
    out, _ = _run(q, k, v)
    return out


# revision 49
# speedup vs baseline: 1.1969x; 1.0209x over previous
"""Causal GQA attention on 8 TRN2 NeuronCores.

Problem: q [4096, 4096] = [bs*seq, 32 heads * 128], k/v [4096, 1024] =
[bs*seq, 8 kv heads * 128], causal softmax(q k^T / sqrt(128)) v with GQA
(4 query heads per kv head). f32 in/out.

Sharding: 8 cores = 2 batches x 4 head-groups. Each core owns one batch
and 8 query heads / 2 kv heads -- fully local, no collectives. Q and K are
handed to each core pre-permuted to [head_dim, head, seq] (host-side layout
marshalling in the shard step) so the contraction dim is already on
partitions; V is packed host-side with a fused ones column ([V_j | 1],
bf16) as the PV matmul wants it.

The kernel is organized around the ScalarE exp bottleneck (~139k PSUM
columns of exp per core at ~1 elem/cycle/lane): the S^T columns for all
(head, key-block) pairs form one continuous stream packed into rotating
[128, 1536] f32 PSUM regions (3 banks each, 2 regions = 6 banks), so every
exp is a single wide ACTIVATE (N=1536) instead of many narrow ones --
per-instruction overhead (~400ns) amortizes 12x better. The causal mask is
applied AFTER exp by a DVE multiply of each diagonal 128x128 P^T subtile
with a 0/1 lower-triangle (bf16 2x mode), keeping the QK->exp path free of
VectorE and making softmax denominators exact (masked probs are 0).

Per-core pipeline over regions r (91 per core):
  QK(r+2) on PE (K_j-stationary, <=512-wide bank-aligned chunks)
  || exp(r) on ScalarE (one ACTIVATE, scale folded in)
  || triangle masks(r) + paired chain drains on DVE
  || PV chains released 2 regions behind the ACT stream on PE:
       acc[q,0:129] = sum_j P^T_j[:, s-subtile] @ [V_j | 1]
     (P^T-stationary accumulation; two chains share one 1-bank PSUM tile
     and drain with a single DVE copy, so the accumulator WAR has 4 chains
     of slack; normalization runs from SBUF in batches of 4 subtiles).
Chain release is budgeted (~16 PV steps/region) to smooth the diag-block
clustering at head tails into the chain-light early regions of the next
head. DMA loads ride one HWDGE queue in strict first-use order (parallel
queues split HBM bandwidth and starve the critical head-0 pieces); a
~4us burst of dummy matmuls opens the PE HAM clock gate while they land.

Walrus sync-wait limits (1 slot on DMA descriptors and LDWEIGHTS): tiny
PE warmup matmuls absorb each DMA piece's semaphore into PE's vector
clock, injected just before the piece's first use so they never park the
PE FIFO on a DMA that hasn't landed.

No max-subtraction softmax: logits are ~N(0,1) after scale, exp stays in
range; diag-block garbage (upper triangle) is finite and zeroed post-exp.
"""

import numpy as np

P = 128          # partitions / head_dim / key block
SEQ = 2048       # per-core sequence length
H = 8            # query heads per core
KV = 2           # kv heads per core
D = 128          # head dim
NB = SEQ // P    # 16 key blocks (also query subtiles) per head
SCALE = float(D) ** -0.5

REG_W = 1536     # S^T stream region width (3 PSUM banks of f32)
HEAD_W = sum(SEQ - P * j for j in range(NB))          # 17408 cols per head
STREAM_W = H * HEAD_W                                  # 139264 per core
NR = (STREAM_W + REG_W - 1) // REG_W                   # 91 regions

# Regions whose exp runs on the DVE (Schraudolph int-trick, see emit_act)
# instead of the saturated ScalarE. Measured: shifting exp into the DVE
# FIFO delays the PSUM st-region rotation (the DVE also carries chain
# copies/masks) and costs more in ACT-stream stalls than it saves, so the
# offload is disabled; the machinery is kept for experimentation.
DVE_EXP_REGIONS = frozenset()
LOG2E = 1.4426950408889634
SCH_A = float(2 ** 23 * LOG2E) * SCALE        # folds the 1/sqrt(d) scale
SCH_B = float(127 * 2 ** 23 - 550000)         # bias minus spline correction

_NC = None


def _block_base(h, j):
    """Stream position of the first column of (head h, key block j)."""
    return h * HEAD_W + SEQ * j - 64 * j * (j - 1)


def _stream_layout():
    """Build-time bookkeeping: region -> QK segments / diag subtiles, and
    (h, j, s) -> (region, offset) for PV chain stationary slices."""
    segs = [[] for _ in range(NR)]    # (h, j, q0, width, region_off)
    diags = [[] for _ in range(NR)]   # (h, j, region_off)
    for h in range(H):
        for j in range(NB):
            w = SEQ - P * j
            base = _block_base(h, j)
            r0, off0 = divmod(base, REG_W)
            diags[r0].append((h, j, off0))
            c = 0
            while c < w:
                r, off = divmod(base + c, REG_W)
                take = min(w - c, REG_W - off, 512 - (off % 512))
                segs[r].append((h, j, P * j + c, take, off))
                c += take
    return segs, diags


def _pt_slice_loc(h, j, s):
    """Region and offset of P^T_j[:, s-th 128-query subtile] for head h."""
    return divmod(_block_base(h, j) + P * (s - j), REG_W)


def _build_nc():
    import concourse.bass as bass
    import concourse.bacc as bacc
    import concourse.mybir as mybir
    import concourse.tile as tile
    from contextlib import ExitStack

    f32 = mybir.dt.float32
    bf16 = mybir.dt.bfloat16
    i32 = mybir.dt.int32
    i16 = mybir.dt.int16
    Exp = mybir.ActivationFunctionType.Exp
    Alu = mybir.AluOpType

    segs, diags = _stream_layout()

    nc = bacc.Bacc()
    qT_ext = nc.declare_dram_parameter("qT", [P, H, SEQ], bf16, isOutput=False)
    kT_ext = nc.declare_dram_parameter("kT", [P, KV, SEQ], bf16, isOutput=False)
    # vones ships pre-marshalled partition-major so the load is one DMA of
    # contiguous 8.25KB per partition (258-byte-descriptor loads take ~10us)
    v_ext = nc.declare_dram_parameter("vones", [P, NB * KV * (D + 1)], bf16,
                                      isOutput=False)
    tri_ext = nc.declare_dram_parameter("tri01", [P, P], bf16, isOutput=False)
    o_ext = nc.declare_dram_parameter("out", [SEQ, H * D], f32, isOutput=True)

    vd = v_ext.rearrange("p (i k c) -> p i k c", i=NB, k=KV)
    od = o_ext.rearrange("(i p) c -> p i c", p=P)

    with ExitStack() as ctx:
        tc = ctx.enter_context(tile.TileContext(nc))
        singles = ctx.enter_context(tc.tile_pool(name="singles", bufs=1))
        pt_pool = ctx.enter_context(tc.tile_pool(name="pt", bufs=20))
        ob_pool = ctx.enter_context(tc.tile_pool(name="ob", bufs=2))
        r_pool = ctx.enter_context(tc.tile_pool(name="r", bufs=8))
        z_pool = ctx.enter_context(tc.tile_pool(name="z", bufs=2))
        ps_st = ctx.enter_context(tc.tile_pool(name="ps_st", bufs=2, space="PSUM"))
        ps_pv = ctx.enter_context(tc.tile_pool(name="ps_pv", bufs=2, space="PSUM"))

        # ---- upfront loads, each into a fresh buffer on a fresh queue ----
        # Head 0 / kv 0 pieces come first so compute starts early.
        kt = singles.tile([P, KV, SEQ], bf16)        # [d, kv, key]
        qt = singles.tile([P, H, SEQ], bf16)         # [d, head, query]
        vones = singles.tile([P, NB, KV, D + 1], bf16)  # [k, block, kv, d|1]
        # Loads are split across the SP (sync) and GpSimd trigger queues:
        # each DMA trigger costs ~0.7us of serial queue time, so the
        # critical pieces (triangle, kt/qt of head 0) sit at the front of
        # one queue while vones rides the other.
        # One queue, strict need-order: concurrent queues split HBM
        # bandwidth and starve the critical head-0 pieces. vones rides after
        # kt0b -- the budgeted chain backlog absorbs its late arrival.
        tri01 = singles.tile([P, P], bf16)
        nc.sync.dma_start(out=tri01, in_=tri_ext.ap())
        nc.sync.dma_start(out=kt[:, 0, 0:128], in_=kT_ext.ap()[:, 0, 0:128])
        nc.sync.dma_start(out=qt[:, 0, 0:1536], in_=qT_ext.ap()[:, 0, 0:1536])
        nc.sync.dma_start(out=kt[:, 0, 128:512], in_=kT_ext.ap()[:, 0, 128:512])
        nc.sync.dma_start(out=qt[:, 0, 1536:], in_=qT_ext.ap()[:, 0, 1536:])
        nc.sync.dma_start(out=kt[:, 0, 512:], in_=kT_ext.ap()[:, 0, 512:])
        nc.sync.dma_start(out=vones, in_=vd)
        nc.sync.dma_start(out=qt[:, 1:4, :], in_=qT_ext.ap()[:, 1:4, :])
        nc.sync.dma_start(out=kt[:, 1:2, :], in_=kT_ext.ap()[:, 1:2, :])
        nc.sync.dma_start(out=qt[:, 4:8, :], in_=qT_ext.ap()[:, 4:8, :])

        # Scratch initialized instantly by DVE at t=0: the exp table-load
        # warmup and HAM pre-warm run on it with no DMA dependency.
        scratch = singles.tile([P, P], f32)
        nc.vector.memset(scratch, 0.5)

        # ---- PE warmups: absorb DMA/DVE semaphores into PE's clock so real
        # matmuls never carry a second wait. Outputs unread.
        def warm(ap):
            wm = ps_pv.tile([2, 2], f32, tag="pvacc", name="wm")
            nc.tensor.matmul(wm[:1, :1], lhsT=ap, rhs=ap, start=True, stop=True)

        # HAM pre-warm: ~2.5us of back-to-back dummy matmuls so the PE clock
        # gate opens before the first real QK burst instead of ~20us in.
        # These run while the DMAs land.
        hamwm = ps_pv.tile([P, P], f32, tag="pvacc", name="hamwm")
        scr16 = scratch.bitcast(bf16)
        for _ in range(48):
            nc.tensor.matmul(hamwm, lhsT=scr16[:, 0:P], rhs=scr16[:, 0:P],
                             start=True, stop=True)

        warm(tri01[:, 0:1])
        warm(kt[:, 0, 0:1])
        warm(qt[:, 0, 0:1])
        # exp table load early, overlapping the remaining DMAs
        actwarm = singles.tile([P, P], bf16)
        nc.scalar.activation(out=actwarm, in_=scratch, func=Exp, scale=SCALE)

        # one warm per remaining DMA piece, injected just before first use
        # (an upfront warm would park the PE FIFO until that DMA lands)
        warm_aps = {
            "q0b": qt[:, 0, 1536:1537],
            "q123": qt[:, 1, 0:1],
            "q4567": qt[:, 4, 0:1],
            "k0a2": kt[:, 0, 128:129],
            "k0b": kt[:, 0, 512:513],
            "k1": kt[:, 1, 0:1],
            "vones": vones[:, 0, 0, 0:1],
        }
        warmed = set()

        def warm_piece(piece):
            if piece and piece not in warmed:
                warmed.add(piece)
                warm(warm_aps[piece])

        def q_piece(h, q1):
            if h == 0:
                return "q0b" if q1 > 1536 else None
            return "q123" if h < 4 else "q4567"

        def k_piece(kvh, j):
            if kvh == 0:
                if j == 0:
                    return None
                return "k0a2" if j < 4 else "k0b"
            return "k1"

        # ---- pipelined region loop ----
        st_tiles = {}
        pt_tiles = {}
        o_sbs = {}

        def emit_qk(r):
            st = ps_st.tile([P, REG_W], f32, name="st")
            st_tiles[r] = st
            for (h, j, q0, w, off) in segs[r]:
                kvh = h // (H // KV)
                warm_piece(q_piece(h, q0 + w))
                warm_piece(k_piece(kvh, j))
                nc.tensor.matmul(
                    st[:, off:off + w],
                    lhsT=kt[:, kvh, j * P:(j + 1) * P],
                    rhs=qt[:, h, q0:q0 + w],
                    start=True,
                    stop=True,
                )

        def emit_act(r):
            w = min(REG_W, STREAM_W - r * REG_W)
            pt = pt_pool.tile([P, REG_W], bf16, name="pt")
            pt_tiles[r] = pt
            st = st_tiles.pop(r)
            if r in DVE_EXP_REGIONS:
                # exp on the DVE: z = round(S * 2^23 log2(e) / sqrt(d) + B)
                # as int32; the high 16 bits of z ARE the bf16 pattern of
                # ~exp(S/sqrt(d)) (linear-mantissa approximation).
                z = z_pool.tile([P, REG_W], i32, name="z")
                nc.vector.tensor_scalar(
                    out=z[:, 0:w], in0=st[:, 0:w],
                    scalar1=SCH_A, scalar2=SCH_B,
                    op0=Alu.mult, op1=Alu.add,
                )
                z_hi = z[:, 0:w].bitcast(i16).rearrange(
                    "p (w two) -> p w two", two=2)[:, :, 1]
                nc.vector.tensor_copy(out=pt[:, 0:w].bitcast(i16), in_=z_hi)
            else:
                nc.scalar.activation(
                    out=pt[:, 0:w], in_=st[:, 0:w], func=Exp, scale=SCALE,
                )

        def emit_masks(r):
            pt = pt_tiles[r]
            for (h, j, off) in diags[r]:
                nc.vector.tensor_mul(
                    out=pt[:, off:off + P],
                    in0=pt[:, off:off + P],
                    in1=tri01,
                )

        # PV accumulators are packed three-to-a-bank: chains (h, s), s
        # consecutive, write thirds of one [P, 3, 129] PSUM tile, drained by
        # a single DVE copy + normalize + store. PE never writes a bank the
        # DVE still reads (group-granularity rotation), drains shrink, and
        # the acc-WAR slack grows to 6 chains.
        GRP = 3
        pair_state = {}  # "tile" -> open acc-group tile

        def flush_group(h, s_hi, n):
            acc = pair_state.pop("tile")
            o_raw = o_sbs[h]
            s_lo = s_hi - n + 1
            nc.vector.tensor_copy(out=o_raw[:, s_lo:s_hi + 1, :],
                                  in_=acc[:, 0:n, :])
            rcp = r_pool.tile([P, 4], f32, name="rcp")
            nc.vector.reciprocal(rcp[:, 0:n], o_raw[:, s_lo:s_hi + 1, D])
            for i in range(n):
                si = s_lo + i
                nc.vector.tensor_scalar_mul(
                    o_raw[:, si, 0:D], o_raw[:, si, 0:D], rcp[:, i:i + 1]
                )
            nc.sync.dma_start(
                out=od[:, s_lo:s_hi + 1, h * D:(h + 1) * D],
                in_=o_raw[:, s_lo:s_hi + 1, 0:D],
            )

        def emit_chain(h, s):
            kvh = h // (H // KV)
            warm_piece("vones")
            if s == 0:
                o_sbs[h] = ob_pool.tile([P, NB, D + 1], f32, name="o_raw")
            if "tile" not in pair_state:
                pair_state["tile"] = ps_pv.tile([P, GRP, D + 1], f32,
                                                tag="pvacc", name="pvacc")
            acc = pair_state["tile"][:, s % GRP, :]
            for j in range(s + 1):
                rr, off = _pt_slice_loc(h, j, s)
                nc.tensor.matmul(
                    acc,
                    lhsT=pt_tiles[rr][:, off:off + P],
                    rhs=vones[:, j, kvh, :],
                    start=(j == 0),
                    stop=(j == s),
                )
            if s % GRP == GRP - 1 or s == NB - 1:
                flush_group(h, s, s % GRP + 1)

        # Iteration r: ACT(r) [needs QK(r), emitted 2 iters ago]; chains for
        # diag region r-1 (runnable: only need ACT(r-1)+mask(r-1)) BEFORE
        # QK(r+2) (gated on ACT(r) via the st WAR) so the PE FIFO never
        # parks runnable chain work behind the region gate. On the DVE FIFO
        # the chain copies/normalizes (gated only on PE) come BEFORE
        # masks(r) (gated on ACT(r)) so they never wait behind it.
        # Chain release is smoothed: diagonal blocks cluster at head tails
        # (block widths shrink toward j=15, so several long chains become
        # runnable in the last regions of a head). A pending queue caps the
        # chain work emitted per iteration (~BUDGET PV steps ~= one ACTIVATE
        # of PE time) and spills the excess into the chain-light early
        # regions of the next head.
        # Chains release 2 regions behind the ACT stream (the PE runs ~1
        # region ahead, so the diag subtile's DVE mask is already done when
        # the chain's last step loads it); at the very tail they release as
        # fresh as possible.
        BUDGET = 16
        pending = []
        released = 0
        for rr in range(min(2, NR)):
            emit_qk(rr)
        for r in range(NR):
            emit_act(r)
            target = r - 1 if r >= NR - 2 else r - 2
            while released <= target:
                pending.extend((h, j) for (h, j, _off) in diags[released])
                released += 1
            steps = 0
            budget = BUDGET if r < NR - 8 else 10 ** 9
            while pending and steps < budget:
                h, s = pending.pop(0)
                emit_chain(h, s)
                steps += s + 1
            emit_masks(r)
            if r + 2 < NR:
                emit_qk(r + 2)
        while released < NR:
            pending.extend((h, j) for (h, j, _off) in diags[released])
            released += 1
        for (h, s) in pending:
            emit_chain(h, s)

    nc.compile()
    return nc


def _get_nc():
    global _NC
    if _NC is None:
        _NC = _build_nc()
    return _NC


def _shard_inputs(q, k, v):
    import ml_dtypes
    in_maps = []
    ones = np.ones((SEQ, KV, 1), np.float32)
    # keep P^T[k, q_local] where q_local >= k
    tri01 = np.triu(np.ones((P, P), np.float32)).astype(ml_dtypes.bfloat16)
    for c in range(8):
        b, hg = divmod(c, 4)
        rs = slice(b * SEQ, (b + 1) * SEQ)
        qs = q[rs, hg * 1024:(hg + 1) * 1024]    # [seq, 8*128]
        ks = k[rs, hg * 256:(hg + 1) * 256]      # [seq, 2*128]
        vs = v[rs, hg * 256:(hg + 1) * 256].reshape(SEQ, KV, D)
        # partition-major: vo[p, i, kv, d|1] for seq = i*128 + p
        vo = np.concatenate([vs, ones], axis=2).reshape(NB, P, KV * (D + 1))
        vo = vo.transpose(1, 0, 2).reshape(P, NB * KV * (D + 1))
        in_maps.append({
            "qT": np.ascontiguousarray(
                qs.reshape(SEQ, H, D).transpose(2, 1, 0)
            ).astype(ml_dtypes.bfloat16),
            "kT": np.ascontiguousarray(
                ks.reshape(SEQ, KV, D).transpose(2, 1, 0)
            ).astype(ml_dtypes.bfloat16),
            "vones": np.ascontiguousarray(vo).astype(ml_dtypes.bfloat16),
            "tri01": tri01,
        })
    return in_maps


def _run(q, k, v, **spmd_kwargs):
    from concourse.bass_utils import run_bass_kernel_spmd

    nc = _get_nc()
    bkr = run_bass_kernel_spmd(nc, _shard_inputs(q, k, v),
                               core_ids=list(range(8)), **spmd_kwargs)
    out = np.empty((2 * SEQ, 32 * D), np.float32)
    for c in range(8):
        b, hg = divmod(c, 4)
        out[b * SEQ:(b + 1) * SEQ, hg * 1024:(hg + 1) * 1024] = \
            bkr.results[c]["out"]
    return out, bkr


def kernel(q, k, v, bs=2, seq_len=2048, **_ignored):
    q = np.asarray(q, dtype=np.float32)
    k = np.asarray(k, dtype=np.float32)
    v = np.asarray(v, dtype=np.float32)
    assert int(bs) == 2 and int(seq_len) == SEQ
    assert q.shape == (4096, 4096) and k.shape == (4096, 1024)
    out, _ = _run(q, k, v)
    return out
